# revision 1
# baseline (speedup 1.0000x reference)
"""Bass/Trainium2 kernel for nn_GCL_49959059587771 (GCL JSD loss).

Math: reference computes, for pair (z, g, batch):
    zn = z / max(||z||, eps);  gn = g / max(||g||, eps)
    self_sim  = (zn @ gn_self.T)  * onehot(batch)   # [N, G]
    cross_sim = (zn @ gn_cross.T) * onehot(batch)
    d = ep_jsd(self_sim).sum(1) - ep_jsd(cross_sim).sum(1)
    L = sqrt(sum(d^2))
where ep_jsd(x) = log2 - softplus(-x), and ep_jsd(0) = 0 exactly. The
one-hot mask therefore collapses each row of the [N, G] matrices to a
single entry: the masked row-sum of ep_jsd equals ep_jsd of the one
gathered dot product (all other entries are ep_jsd(0) = 0, and even their
shared constant would cancel in the self-cross difference).

So per node i:
    s_i = <z_i, gn_self[b_i]>  / ||z_i||
    c_i = <z_i, gn_cross[b_i]> / ||z_i||
    d_i = softplus(-c_i) - softplus(-s_i)
and the answer is sqrt(sum d1^2) + sqrt(sum d2^2).

Strategy (8 NeuronCores, SPMD, node-parallel):
  - shard nodes N across 8 cores (6250 each, padded to 6272 = 49*128)
  - replicate g (concatenated [g1 | g2] rows -> one 512-col "cat row")
  - on device: normalize g rows once, store to DRAM scratch, then
    dma_gather cat rows per node; per 128-node tile use the fused DVE
    tensor_tensor_reduce (multiply + free-axis reduce) for the two dot
    products and ACT Square+accum for ||z||^2; tiny [128, 49] epilogue
    does the normalize / softplus / d^2 accumulation via Exp/Ln.
  - per-core output: [128, 2] partial sums of d1^2 / d2^2; host finishes
    the all-reduce (sum over cores+partitions), sqrt, add.
"""

import numpy as np
from contextlib import ExitStack

import concourse.bass as bass
import concourse.bacc as bacc
import concourse.tile as tile
import concourse.mybir as mybir
from concourse.bass_utils import run_bass_kernel_spmd

N, G, D = 50000, 512, 256
NCORES = 8
RPC = N // NCORES            # 6250 rows per core
NT = 49                      # 128-row tiles per core
RPAD = NT * 128              # 6272
GRP = 7                      # tiles per gather/DMA group
NGRP = NT // GRP             # 7

AF = mybir.ActivationFunctionType
ALU = mybir.AluOpType
F32 = mybir.dt.float32
I16 = mybir.dt.int16
BF16 = mybir.dt.bfloat16

# compute dtype for z tiles and normalized-g gather payloads
Z_DT = BF16
G_DT = BF16
_NP_Z = {F32: np.float32, BF16: "bfloat16"}


def build(z_dt=Z_DT, g_dt=G_DT, debug=False):
    nc = bacc.Bacc("TRN2", target_bir_lowering=False, debug=debug)

    z1t = nc.dram_tensor("z1t", [128, NT, D], z_dt, kind="ExternalInput")
    z2t = nc.dram_tensor("z2t", [128, NT, D], z_dt, kind="ExternalInput")
    # one-hot routing matrices: oh[v_local, t, p] = 1 iff node t*128+p has
    # (windowed) batch value v_local.  Gather becomes OH.T @ Gwin on TensorE.
    oh1 = nc.dram_tensor("oh1", [128, NT, 128], g_dt, kind="ExternalInput")
    oh2 = nc.dram_tensor("oh2", [128, NT, 128], g_dt, kind="ExternalInput")
    # per-core 128-value window of [g1 | g2] cat rows (raw; device normalizes)
    gwin = nc.dram_tensor("gwin", [128, 2 * D], F32, kind="ExternalInput")
    acc = nc.dram_tensor("acc", [128, 2], F32, kind="ExternalOutput")

    with tile.TileContext(nc) as tc, ExitStack() as ctx:
        gpool = ctx.enter_context(tc.tile_pool(name="gnorm", bufs=2))
        singles = ctx.enter_context(tc.tile_pool(name="singles", bufs=1))
        zpool = ctx.enter_context(tc.tile_pool(name="z", bufs=3))
        ggpool = ctx.enter_context(tc.tile_pool(name="gg", bufs=4,
                                                space="PSUM"))
        junk = ctx.enter_context(tc.tile_pool(name="junk", bufs=6))
        small = ctx.enter_context(tc.tile_pool(name="small", bufs=4))

        # ---- phase 1: row-normalize the g window; keep resident in SBUF ----
        g_in = gpool.tile([128, 2 * D], F32, tag="g_in")
        nc.sync.dma_start(g_in[:], gwin[:])
        gn2 = small.tile([128, 2], F32, tag="gn2")
        for h in range(2):
            sq = junk.tile([128, D], F32, tag="junk")
            nc.vector.scalar_tensor_tensor(
                out=sq[:], in0=g_in[:, h * D:(h + 1) * D], scalar=1.0,
                in1=g_in[:, h * D:(h + 1) * D],
                op0=ALU.mult, op1=ALU.mult, accum_out=gn2[:, h:h + 1])
        # 1/sqrt(n2) = exp(-0.5 * ln(n2)); g norms are O(16), no eps issue
        ginv = small.tile([128, 2], F32, tag="ginv")
        nc.scalar.activation(out=ginv[:], in_=gn2[:], func=AF.Ln)
        nc.scalar.activation(out=ginv[:], in_=ginv[:], func=AF.Exp, scale=-0.5)
        gnorm = singles.tile([128, 2 * D], g_dt)
        for h in range(2):
            nc.vector.tensor_scalar_mul(
                gnorm[:, h * D:(h + 1) * D], g_in[:, h * D:(h + 1) * D],
                ginv[:, h:h + 1])

        # ---- per-tile accumulators ----
        r1s = singles.tile([128, NT], F32)
        r1c = singles.tile([128, NT], F32)
        r2s = singles.tile([128, NT], F32)
        r2c = singles.tile([128, NT], F32)
        nn1 = singles.tile([128, NT], F32)
        nn2 = singles.tile([128, NT], F32)

        # ---- main loop ----
        for grp in range(NGRP):
            z1c = zpool.tile([128, GRP, D], z_dt, tag="z1c")
            nc.sync.dma_start(z1c[:], z1t[:, grp * GRP:(grp + 1) * GRP, :])
            z2c = zpool.tile([128, GRP, D], z_dt, tag="z2c")
            nc.sync.dma_start(z2c[:], z2t[:, grp * GRP:(grp + 1) * GRP, :])
            oh1c = zpool.tile([128, GRP, 128], g_dt, tag="oh1c")
            nc.sync.dma_start(oh1c[:], oh1[:, grp * GRP:(grp + 1) * GRP, :])
            oh2c = zpool.tile([128, GRP, 128], g_dt, tag="oh2c")
            nc.sync.dma_start(oh2c[:], oh2[:, grp * GRP:(grp + 1) * GRP, :])
            for tt in range(GRP):
                t = grp * GRP + tt
                for (zc, ohc, rs, rc, nn) in ((z1c, oh1c, r1s, r1c, nn1),
                                              (z2c, oh2c, r2s, r2c, nn2)):
                    # gather normalized cat rows: gg = OH.T @ gnorm (PSUM)
                    gg = ggpool.tile([128, 2 * D], F32, tag="gg")
                    nc.tensor.matmul(gg[:], ohc[:, tt, :], gnorm[:],
                                     start=True, stop=True)
                    # self dot: pair 1 uses g1n (cols 0:D), pair 2 uses g2n
                    sh, ch = (0, D) if zc is z1c else (D, 0)
                    js = junk.tile([128, D], z_dt, tag="junk")
                    nc.vector.scalar_tensor_tensor(
                        out=js[:], in0=zc[:, tt, :], scalar=1.0,
                        in1=gg[:, sh:sh + D],
                        op0=ALU.mult, op1=ALU.mult, accum_out=rs[:, t:t + 1])
                    jc = junk.tile([128, D], z_dt, tag="junk")
                    nc.vector.scalar_tensor_tensor(
                        out=jc[:], in0=zc[:, tt, :], scalar=1.0,
                        in1=gg[:, ch:ch + D],
                        op0=ALU.mult, op1=ALU.mult, accum_out=rc[:, t:t + 1])
                    jn = junk.tile([128, D], F32, tag="junk")
                    nc.scalar.activation(out=jn[:], in_=zc[:, tt, :],
                                         func=AF.Square,
                                         accum_out=nn[:, t:t + 1])

        # ---- epilogue on [128, NT] ----
        # inv_norm = exp(-0.5*ln(n2 + eps));  eps keeps padded zero rows finite
        eps_b = singles.tile([128, 1], F32)
        nc.vector.memset(eps_b[:], 1e-12)
        inv1 = singles.tile([128, NT], F32)
        nc.scalar.activation(out=inv1[:], in_=nn1[:], func=AF.Ln, bias=eps_b[:])
        nc.scalar.activation(out=inv1[:], in_=inv1[:], func=AF.Exp, scale=-0.5)
        inv2 = singles.tile([128, NT], F32)
        nc.scalar.activation(out=inv2[:], in_=nn2[:], func=AF.Ln, bias=eps_b[:])
        nc.scalar.activation(out=inv2[:], in_=inv2[:], func=AF.Exp, scale=-0.5)

        acc_sb = singles.tile([128, 2], F32)
        for j, (rs, rc, inv) in enumerate(((r1s, r1c, inv1), (r2s, r2c, inv2))):
            s = small.tile([128, NT], F32, tag="s")
            nc.vector.tensor_mul(s[:], rs[:], inv[:])
            c = small.tile([128, NT], F32, tag="c")
            nc.vector.tensor_mul(c[:], rc[:], inv[:])
            # softplus(-x) = ln(1 + exp(-x))
            sp_s = small.tile([128, NT], F32, tag="sp_s")
            nc.scalar.activation(out=sp_s[:], in_=s[:], func=AF.Exp, scale=-1.0)
            nc.scalar.activation(out=sp_s[:], in_=sp_s[:], func=AF.Ln, bias=1.0)
            sp_c = small.tile([128, NT], F32, tag="sp_c")
            nc.scalar.activation(out=sp_c[:], in_=c[:], func=AF.Exp, scale=-1.0)
            nc.scalar.activation(out=sp_c[:], in_=sp_c[:], func=AF.Ln, bias=1.0)
            d = small.tile([128, NT], F32, tag="d")
            nc.vector.tensor_sub(d[:], sp_c[:], sp_s[:])
            jd = junk.tile([128, NT], F32, tag="jd")
            nc.scalar.activation(out=jd[:], in_=d[:], func=AF.Square,
                                 accum_out=acc_sb[:, j:j + 1])
        nc.sync.dma_start(acc[:], acc_sb[:])

    nc.compile()
    return nc


# ---------------------------------------------------------------------------
# Scheme X: transposed-z, fully matmul-based variant.
#
# Layouts per core (nodes padded to NODES = 13*512 = 6656, chunks of 512):
#   zT[j, p, i]   : [2, 128, NODES] bf16, element = z[node i, d = j*128+p]
#   ohd[v, i]     : [128, NODES] bf16, rows 0:64 one-hot of (b - v0[half]),
#                   rows 64:128 duplicate (for the cross-g half of P_cat)
#   gs[w, :, :]   : [4, 128, D] f32, w = pair*2 + half; rows 0:64 raw g1
#                   window rows, 64:128 raw g2 window rows (pad rows = 1.0)
#   sel[p, 0:2]   : [128, 2] bf16, col0 = 1_{p<64}, col1 = 1_{p>=64}
#   ones[p, 0:1]  : [128, 1] bf16 all-ones
#   ident         : [128, 128] bf16 identity (PE transpose helper)
#
# Per chunk c (pair p12, w = p12*2 + half(c)):
#   P_cat[vcat, i] = sum_d gsT[w][d, vcat] * zT[d, i]      (2 matmuls, PSUM)
#   masked = (ohd_chunk * inv_cat[w]) * P_cat              (1 DVE stt, SBUF)
#   s/c rows = sel.T @ masked                              (1 matmul -> SCN)
#   n row    = ones.T @ (zT_chunk^2)                       (2 matmuls -> SCN)
# SCN bank packing (one [128, 1024] PSUM tile = 2 banks per chunk):
#   s1@p0 c1@p1 s2@p64 c2@p65 cols 0:512 ; n1@p32 n2@p96 cols 512:1024
# Evac: DVE copy rows {0,32,64,96} x 1024 + ACT copy rows {1,65} x 512
# into SBUF stages; per-chunk DRAM dump; strided reshape loads produce
# [128, 52] natural-layout s/c/n for the same epilogue as v1.
# ---------------------------------------------------------------------------

NODES = 6656                 # padded nodes per core (13 chunks of 512)
NCH = NODES // 512           # 13
HALF_CH = 7                  # chunks 0:7 -> half A, 7:13 -> half B
WCOLS = NCH * 4              # 52 columns in reshaped [128, 52] layout


def build_x(z_dt=Z_DT, g_dt=G_DT, debug=False):
    nc = bacc.Bacc("TRN2", target_bir_lowering=False, debug=debug)

    zT1 = nc.dram_tensor("zT1", [2, 128, NODES], z_dt, kind="ExternalInput")
    zT2 = nc.dram_tensor("zT2", [2, 128, NODES], z_dt, kind="ExternalInput")
    ohd1 = nc.dram_tensor("ohd1", [128, NODES], g_dt, kind="ExternalInput")
    ohd2 = nc.dram_tensor("ohd2", [128, NODES], g_dt, kind="ExternalInput")
    gs = nc.dram_tensor("gs", [4, 128, D], F32, kind="ExternalInput")
    sel = nc.dram_tensor("sel", [128, 2], g_dt, kind="ExternalInput")
    ones = nc.dram_tensor("ones", [128, 1], g_dt, kind="ExternalInput")
    ident = nc.dram_tensor("ident", [128, 128], F32, kind="ExternalInput")
    scratch = nc.dram_tensor("scratch", [6, NODES], F32)
    acc = nc.dram_tensor("acc", [128, 2], F32, kind="ExternalOutput")

    with tile.TileContext(nc) as tc, ExitStack() as ctx:
        singles = ctx.enter_context(tc.tile_pool(name="singles", bufs=1))
        zpool = ctx.enter_context(tc.tile_pool(name="z", bufs=3))
        junk = ctx.enter_context(tc.tile_pool(name="junk", bufs=4))
        small = ctx.enter_context(tc.tile_pool(name="small", bufs=4))
        ppool = ctx.enter_context(tc.tile_pool(name="pp", bufs=3, space="PSUM"))
        spool = ctx.enter_context(tc.tile_pool(name="sp", bufs=3, space="PSUM"))
        tpool = ctx.enter_context(tc.tile_pool(name="tp", bufs=2, space="PSUM"))

        sel_sb = singles.tile([128, 2], g_dt)
        nc.sync.dma_start(sel_sb[:], sel[:])
        ones_sb = singles.tile([128, 1], g_dt)
        nc.sync.dma_start(ones_sb[:], ones[:])
        id_sb = singles.tile([128, 128], F32)
        nc.sync.dma_start(id_sb[:], ident[:])

        # ---- phase 1: per-window inv norms + transposed raw g (bf16) ----
        inv_cat = singles.tile([128, 4], F32)
        gT = singles.tile([128, 4, 2, 128], g_dt)
        for w in range(4):
            gw = zpool.tile([128, D], F32, tag="gw")
            nc.sync.dma_start(gw[:], gs[w, :, :])
            n2 = small.tile([128, 1], F32, tag="gn2")
            sq = junk.tile([128, D], F32, tag="junk")
            nc.vector.scalar_tensor_tensor(
                out=sq[:], in0=gw[:], scalar=1.0, in1=gw[:],
                op0=ALU.mult, op1=ALU.mult, accum_out=n2[:])
            nc.scalar.activation(out=inv_cat[:, w:w + 1], in_=n2[:], func=AF.Ln)
            nc.scalar.activation(out=inv_cat[:, w:w + 1],
                                 in_=inv_cat[:, w:w + 1],
                                 func=AF.Exp, scale=-0.5)
            for k in range(2):
                tp = tpool.tile([128, 128], F32, tag="gtp")
                nc.tensor.transpose(tp[:], gw[:, k * 128:(k + 1) * 128],
                                    id_sb[:])
                nc.vector.tensor_copy(gT[:, w, k, :], tp[:])

        # ---- main loop over 512-node chunks ----
        stA = singles.tile([4, NCH, 1024], F32)
        stB = singles.tile([2, NCH, 512], F32)
        for c in range(NCH):
            h = 0 if c < HALF_CH else 1
            cs = slice(c * 512, (c + 1) * 512)
            z1c = zpool.tile([128, 2, 512], z_dt, tag="z1c")
            nc.sync.dma_start(z1c[:, 0, :], zT1[0, :, cs])
            nc.sync.dma_start(z1c[:, 1, :], zT1[1, :, cs])
            z2c = zpool.tile([128, 2, 512], z_dt, tag="z2c")
            nc.sync.dma_start(z2c[:, 0, :], zT2[0, :, cs])
            nc.sync.dma_start(z2c[:, 1, :], zT2[1, :, cs])
            oh1c = zpool.tile([128, 512], g_dt, tag="oh1c")
            nc.sync.dma_start(oh1c[:], ohd1[:, cs])
            oh2c = zpool.tile([128, 512], g_dt, tag="oh2c")
            nc.sync.dma_start(oh2c[:], ohd2[:, cs])

            scn = spool.tile([128, 1024], F32, tag="scn")
            for p12 in range(2):
                w = p12 * 2 + h
                zc = z1c if p12 == 0 else z2c
                ohc = oh1c if p12 == 0 else oh2c
                pcat = ppool.tile([128, 512], F32, tag="pcat")
                for k in range(2):
                    nc.tensor.matmul(pcat[:], gT[:, w, k, :], zc[:, k, :],
                                     start=(k == 0), stop=(k == 1))
                masked = junk.tile([128, 512], g_dt, tag="masked")
                nc.vector.scalar_tensor_tensor(
                    out=masked[:], in0=ohc[:], scalar=inv_cat[:, w:w + 1],
                    in1=pcat[:], op0=ALU.mult, op1=ALU.mult)
                # s,c rows at partitions {0,1} / {64,65}
                nc.tensor.matmul(scn[p12 * 64:p12 * 64 + 2, 0:512],
                                 sel_sb[:], masked[:], start=True, stop=True,
                                 tile_position=(0, p12 * 64))
                # norm row at partition {32} / {96}, cols 512:1024
                zsq = junk.tile([128, 2, 512], z_dt, tag="zsq")
                nc.scalar.activation(out=zsq[:, 0, :], in_=zc[:, 0, :],
                                     func=AF.Square)
                nc.scalar.activation(out=zsq[:, 1, :], in_=zc[:, 1, :],
                                     func=AF.Square)
                for k in range(2):
                    nc.tensor.matmul(
                        scn[p12 * 64 + 32:p12 * 64 + 33, 512:1024],
                        ones_sb[:], zsq[:, k, :],
                        start=(k == 0), stop=(k == 1),
                        tile_position=(0, p12 * 64 + 32))
            # evacuate: rows {0,32,64,96} x 1024 (s1,n1,s2,n2), rows {1,65}
            evA = bass.AP(tensor=scn.tensor, offset=scn.offset,
                          ap=[[32 * scn.ap[0][0], 4]] + scn.ap[1:]) \
                if False else scn[:]
            nc.vector.tensor_copy(stA[:, c, :], scn[0:97:32, :])
            nc.scalar.copy(stB[:, c, :], scn[1:66:64, 0:512])
            nc.sync.dma_start(scratch[0:4, cs.start * 2:cs.stop * 2]
                              if False else scratch[0:1, 0:1], stA[0:1, c, 0:1])

        nc.compile()
    return nc


_prog = None


def _get_prog():
    global _prog
    if _prog is None:
        _prog = build()
    return _prog


def _prep_inputs(z1, z2, g1, g2, batch_1, batch_2):
    import ml_dtypes  # noqa: F401  (registers bfloat16 with numpy)
    z1 = np.asarray(z1, dtype=np.float32)
    z2 = np.asarray(z2, dtype=np.float32)
    b1 = np.asarray(batch_1).astype(np.int64).ravel()
    b2 = np.asarray(batch_2).astype(np.int64).ravel()
    gcat = np.concatenate([np.asarray(g1, np.float32),
                           np.asarray(g2, np.float32)], axis=1)  # [G, 2D]
    z_np = np.dtype("float32") if Z_DT == F32 else np.dtype("bfloat16")
    g_np = np.dtype("float32") if G_DT == F32 else np.dtype("bfloat16")

    in_maps = []
    for k in range(NCORES):
        sl = slice(k * RPC, (k + 1) * RPC)

        def prep_z(z):
            zs = np.zeros((RPAD, D), np.float32)
            zs[:RPC] = z[sl]
            zt = zs.reshape(NT, 128, D).transpose(1, 0, 2)
            return np.ascontiguousarray(zt.astype(z_np))

        # shared 128-value window for this core (both batches index g rows)
        v0 = int(min(b1[sl].min(), b2[sl].min()))
        vhi = int(max(b1[sl].max(), b2[sl].max()))
        assert vhi - v0 < 128, f"core {k}: value span {vhi - v0 + 1} > 128"
        gw = np.zeros((128, 2 * D), np.float32)
        nrows = min(128, G - v0)
        gw[:nrows] = gcat[v0:v0 + nrows]
        gw[nrows:] = 1.0  # never-selected pad rows; keep norms finite

        def prep_oh(b):
            bl = (b[sl] - v0).astype(np.int64)          # [RPC] in [0,128)
            oh = np.zeros((128, RPAD), np.float32)      # [v_local, node]
            oh[bl, np.arange(RPC)] = 1.0
            oh = oh.reshape(128, NT, 128)
            return np.ascontiguousarray(oh.astype(g_np))

        in_maps.append({"z1t": prep_z(z1), "z2t": prep_z(z2),
                        "oh1": prep_oh(b1), "oh2": prep_oh(b2),
                        "gwin": np.ascontiguousarray(gw)})
    return in_maps


def _finish(results):
    tot = np.zeros(2, np.float64)
    for r in results:
        tot += r["acc"].astype(np.float64).sum(axis=0)
    return np.float32(np.sqrt(tot[0]) + np.sqrt(tot[1]))


def kernel(z1, z2, g1, g2, batch_1, batch_2, trace=False):
    nc = _get_prog()
    in_maps = _prep_inputs(z1, z2, g1, g2, batch_1, batch_2)
    res = run_bass_kernel_spmd(nc, in_maps, core_ids=list(range(NCORES)),
                               trace=trace)
    out = _finish(res.results)
    if trace:
        kernel.last_results = res
    return out



# revision 3
# speedup vs baseline: 1.0672x; 1.0672x over previous
"""Bass/Trainium2 kernel v2 for nn_GCL_49959059587771 (GCL JSD loss).

Math (see reference): per node i and pair p with sorted batch index b_i:
    s_i = <z_i, gn_self[b_i]> / ||z_i||
    c_i = <z_i, gn_cross[b_i]> / ||z_i||
    d_i = softplus(-c_i) - softplus(-s_i)
    answer = sqrt(sum d1_i^2) + sqrt(sum d2_i^2)

v2 strategy (vs v1's per-tile DVE dot products):
  - nodes sharded 8 ways (6250/core, padded to 13 chunks of 512)
  - z shipped TRANSPOSED in fp8: [chunk, d(128 part), kplane(2), node(512)]
  - since batch is sorted, each 512-node chunk's batch values span a tiny
    window (<32).  Per-chunk weights hold the g-rows of both pairs' windows
    (128 cat columns); ONE DoubleRow fp8 matmul per pair per chunk computes
    all window sims: P[v, i] = <g_cat[v], z_i>  (K=256 in one MM).
  - one DVE op per chunk applies the one-hot mask (fused with PSUM evac)
  - per-chunk one-hot reduce-matmuls accumulate s/c rows and (from fp8
    squared z) norm rows into three persistent PSUM banks, laid out
    [26 = pair*13 + chunk, 512] so the epilogue is base-aligned
  - short Ln/Exp epilogue produces per-row sums of d^2; host finishes.
"""

import numpy as np
from contextlib import ExitStack

import concourse.bass as bass
import concourse.bacc as bacc
import concourse.tile as tile
import concourse.mybir as mybir
from concourse.bass_utils import run_bass_kernel_spmd

N, G, D = 50000, 512, 256
NCORES = 8
RPC = N // NCORES            # 6250 nodes per core
CH = 512                     # nodes per chunk
NCH = 13                     # chunks per core
NODES = NCH * CH             # 6656 padded
W = 32                       # per-chunk batch-value window (per pair)

AF = mybir.ActivationFunctionType
ALU = mybir.AluOpType
F32 = mybir.dt.float32
BF16 = mybir.dt.bfloat16
F8 = mybir.dt.float8e4
DR = mybir.MatmulPerfMode.DoubleRow

USE_DR = True


def build(debug=False):
    nc = bacc.Bacc("TRN2", target_bir_lowering=False, debug=debug)

    z1t = nc.dram_tensor("z1t", [NCH, 128, 2, CH], F8, kind="ExternalInput")
    z2t = nc.dram_tensor("z2t", [NCH, 128, 2, CH], F8, kind="ExternalInput")
    # one-hot mask rows: [0:32) pair1 window, [32:64) copy of pair1,
    # [64:96) pair2 window, [96:128) copy of pair2
    oht = nc.dram_tensor("oht", [NCH, 128, CH], F8, kind="ExternalInput")
    # P-matmul weights: cat g rows, transposed: [c, d, k, j].  gw1t: j in
    # [0:32) g1n[v01+j], [32:64) g2n[v01+..], rest 0.  gw2t: [64:96)
    # g2n[v02+..], [96:128) g1n[v02+..], rest 0.  Zero halves let both
    # pairs' MMs write one full PSUM bank via accumulation (DoubleRow
    # can't use col tile_position offsets).
    gw1t = nc.dram_tensor("gw1t", [NCH, 128, 2, 128], F8, kind="ExternalInput")
    gw2t = nc.dram_tensor("gw2t", [NCH, 128, 2, 128], F8, kind="ExternalInput")
    # gather reduce weights (bf16): rows->cols one-hot sums
    rws = nc.dram_tensor("rws", [NCH, 128, 26], BF16, kind="ExternalInput")
    rwc = nc.dram_tensor("rwc", [NCH, 128, 26], BF16, kind="ExternalInput")
    # norm reduce weights (fp8, DoubleRow layout)
    rwn1 = nc.dram_tensor("rwn1", [NCH, 128, 2, 32], F8, kind="ExternalInput")
    rwn2 = nc.dram_tensor("rwn2", [NCH, 128, 2, 32], F8, kind="ExternalInput")
    acc = nc.dram_tensor("acc", [26, 1], F32, kind="ExternalOutput")

    with tile.TileContext(nc) as tc, ExitStack() as ctx:
        singles = ctx.enter_context(tc.tile_pool(name="singles", bufs=1))
        wpool = ctx.enter_context(tc.tile_pool(name="w", bufs=3))
        zpool = ctx.enter_context(tc.tile_pool(name="z", bufs=3))
        sqpool = ctx.enter_context(tc.tile_pool(name="sq", bufs=2))
        mkpool = ctx.enter_context(tc.tile_pool(name="mk", bufs=2))
        ppool = ctx.enter_context(tc.tile_pool(name="pp", bufs=2, space="PSUM"))
        apool = ctx.enter_context(tc.tile_pool(name="acc", bufs=1, space="PSUM"))

        acc_s = apool.tile([26, CH], F32)
        acc_c = apool.tile([26, CH], F32)
        acc_n = apool.tile([32, CH], F32)

        eps_b = singles.tile([26, 1], F32)
        nc.vector.memset(eps_b[:], 1e-12)
        one_b = singles.tile([26, 1], F32)
        nc.vector.memset(one_b[:], 1.0)
        # preload the natural_log_exp ACT table set early (overlaps DMA)
        dummy = singles.tile([26, 1], F32)
        nc.scalar.activation(out=dummy[:], in_=eps_b[:], func=AF.Ln, bias=1.0)

        # software-pipelined issue: stage[c] holds chunk-c tiles to finish
        stage = {}
        for c in range(NCH + 1):
            if c < NCH:
                # ---- DMAs for chunk c ----
                gw1c = wpool.tile([128, 2, 128], F8, tag="gw1")
                nc.sync.dma_start(gw1c[:], gw1t[c])
                gw2c = wpool.tile([128, 2, 128], F8, tag="gw2")
                nc.sync.dma_start(gw2c[:], gw2t[c])
                rwsc = wpool.tile([128, 26], BF16, tag="rws")
                nc.sync.dma_start(rwsc[:], rws[c])
                rwcc = wpool.tile([128, 26], BF16, tag="rwc")
                nc.sync.dma_start(rwcc[:], rwc[c])
                rn1c = wpool.tile([128, 2, 32], F8, tag="rn1")
                nc.sync.dma_start(rn1c[:], rwn1[c])
                rn2c = wpool.tile([128, 2, 32], F8, tag="rn2")
                nc.sync.dma_start(rn2c[:], rwn2[c])
                z1c = zpool.tile([128, 2, CH], F8, tag="z1")
                nc.sync.dma_start(z1c[:], z1t[c])
                z2c = zpool.tile([128, 2, CH], F8, tag="z2")
                nc.sync.dma_start(z2c[:], z2t[c])
                ohc = zpool.tile([128, CH], F8, tag="oh")
                nc.sync.dma_start(ohc[:], oht[c])

                # ---- P matmuls (PE) ----
                pc = ppool.tile([128, CH], F32, tag="p")
                if USE_DR:
                    nc.tensor.matmul(pc[:], gw1c[:], z1c[:],
                                     start=True, stop=False, perf_mode=DR,
                                     skip_group_check=True)
                    nc.tensor.matmul(pc[:], gw2c[:], z2c[:],
                                     start=False, stop=True, perf_mode=DR,
                                     skip_group_check=True)
                else:
                    for k in range(2):
                        nc.tensor.matmul(pc[:], gw1c[:, k, :],
                                         z1c[:, k, :], start=(k == 0),
                                         stop=False)
                        nc.tensor.matmul(pc[:], gw2c[:, k, :],
                                         z2c[:, k, :], start=False,
                                         stop=(k == 1))

                # ---- squares (split ACT / DVE for balance) ----
                zq1 = sqpool.tile([128, 2, CH], F8, tag="q1")
                zq2 = sqpool.tile([128, 2, CH], F8, tag="q2")
                nc.scalar.activation(out=zq1[:], in_=z1c[:], func=AF.Square)
                if c % 2 == 0:
                    nc.scalar.activation(out=zq2[:], in_=z2c[:],
                                         func=AF.Square)
                else:
                    nc.vector.scalar_tensor_tensor(
                        out=zq2[:], in0=z2c[:], scalar=1.0, in1=z2c[:],
                        op0=ALU.mult, op1=ALU.mult)

                # ---- mask + PSUM evac fused (DVE) ----
                mkc = mkpool.tile([128, CH], BF16, tag="mk")
                nc.vector.scalar_tensor_tensor(
                    out=mkc[:], in0=pc[:], scalar=1.0, in1=ohc[:],
                    op0=ALU.mult, op1=ALU.mult)

                stage[c] = (rwsc, rwcc, rn1c, rn2c, mkc, zq1, zq2)

            # ---- reduce matmuls for the PREVIOUS chunk (keeps PE queue
            # from stalling on this chunk's DVE work) ----
            cc = c - 1
            if cc >= 0:
                rwsc, rwcc, rn1c, rn2c, mkc, zq1, zq2 = stage.pop(cc)
                first, last = cc == 0, cc == NCH - 1
                nc.tensor.matmul(acc_s[:], rwsc[:], mkc[:], start=first,
                                 stop=last, skip_group_check=True)
                nc.tensor.matmul(acc_c[:], rwcc[:], mkc[:], start=first,
                                 stop=last, skip_group_check=True)
                if USE_DR:
                    nc.tensor.matmul(acc_n[:], rn1c[:], zq1[:], start=first,
                                     stop=False, perf_mode=DR,
                                     skip_group_check=True)
                    nc.tensor.matmul(acc_n[:], rn2c[:], zq2[:], start=False,
                                     stop=last, perf_mode=DR,
                                     skip_group_check=True)
                else:
                    for k in range(2):
                        nc.tensor.matmul(acc_n[:], rn1c[:, k, :], zq1[:, k, :],
                                         start=(first and k == 0), stop=False,
                                         skip_group_check=True)
                        nc.tensor.matmul(acc_n[:], rn2c[:, k, :], zq2[:, k, :],
                                         start=False, stop=(last and k == 1),
                                         skip_group_check=True)

        # ---- epilogue ----
        inv = singles.tile([26, CH], F32)
        nc.scalar.activation(out=inv[:], in_=acc_n[0:26, :], func=AF.Ln,
                             bias=eps_b[:])
        nc.scalar.activation(out=inv[:], in_=inv[:], func=AF.Exp, scale=-0.5)
        sp = singles.tile([26, CH], F32)
        nc.vector.scalar_tensor_tensor(out=sp[:], in0=acc_s[:], scalar=1.0,
                                       in1=inv[:], op0=ALU.mult, op1=ALU.mult)
        cp = singles.tile([26, CH], F32)
        nc.vector.scalar_tensor_tensor(out=cp[:], in0=acc_c[:], scalar=1.0,
                                       in1=inv[:], op0=ALU.mult, op1=ALU.mult)
        # softplus(-x) = ln(1 + exp(-x)); Exp and Ln share one table set
        nc.scalar.activation(out=sp[:], in_=sp[:], func=AF.Exp, scale=-1.0)
        nc.scalar.activation(out=sp[:], in_=sp[:], func=AF.Ln, bias=one_b[:])
        nc.scalar.activation(out=cp[:], in_=cp[:], func=AF.Exp, scale=-1.0)
        nc.scalar.activation(out=cp[:], in_=cp[:], func=AF.Ln, bias=one_b[:])
        dts = singles.tile([26, CH], F32)
        nc.vector.scalar_tensor_tensor(out=dts[:], in0=cp[:], scalar=1.0,
                                       in1=sp[:], op0=ALU.mult,
                                       op1=ALU.subtract)
        jnk = singles.tile([26, CH], F32)
        accq = singles.tile([26, 1], F32)
        nc.scalar.activation(out=jnk[:], in_=dts[:], func=AF.Square,
                             accum_out=accq[:])
        nc.sync.dma_start(acc[:], accq[:])

    nc.compile()
    return nc


# ---------------------------------------------------------------------------
# host-side prep
# ---------------------------------------------------------------------------

def _prep_core(z1s, z2s, b1s, b2s, g1n, g2n):
    """Build one core's input map. z*s: [RPC, D] f32; b*s sorted int; g*n
    row-normalized [G, D] f32."""
    import ml_dtypes
    f8 = ml_dtypes.float8_e4m3
    bf = ml_dtypes.bfloat16

    def to_f8(x):
        return np.clip(x, -240.0, 240.0).astype(f8)

    nreal = z1s.shape[0]

    z1p = np.zeros((NODES, D), np.float32)
    z1p[:nreal] = z1s
    z2p = np.zeros((NODES, D), np.float32)
    z2p[:nreal] = z2s

    # [NCH, 128, 2, CH] transposed fp8
    def prep_z(zp):
        zt = zp.reshape(NCH, CH, 2, 128).transpose(0, 3, 2, 1)
        return np.ascontiguousarray(to_f8(zt))

    # per-chunk windows
    v01 = np.zeros(NCH, np.int64)
    v02 = np.zeros(NCH, np.int64)
    for c in range(NCH):
        lo = min(c * CH, nreal - 1)
        hi = min((c + 1) * CH, nreal)
        v01[c] = b1s[lo]
        v02[c] = b2s[lo]
        assert b1s[hi - 1] - v01[c] < W, f"chunk {c}: pair1 span too wide"
        assert b2s[hi - 1] - v02[c] < W, f"chunk {c}: pair2 span too wide"

    oh = np.zeros((NCH, 128, CH), np.float32)
    for c in range(NCH):
        hi = min((c + 1) * CH, nreal)
        nn = hi - c * CH
        if nn <= 0:
            continue
        idx = np.arange(nn)
        r1 = (b1s[c * CH:hi] - v01[c]).astype(np.int64)
        r2 = (b2s[c * CH:hi] - v02[c]).astype(np.int64)
        oh[c, r1, idx] = 1.0
        oh[c, W + r1, idx] = 1.0
        oh[c, 2 * W + r2, idx] = 1.0
        oh[c, 3 * W + r2, idx] = 1.0

    # gw1/gw2: [NCH, 128, 2, 128] fp8; column blocks of W rows from g
    # windows; unused halves zero (see dram tensor comment).
    # z layout packs d as (kplane, part): d = k*128 + p, k outer.
    gw1 = np.zeros((NCH, 128, 2, 128), np.float32)
    gw2 = np.zeros((NCH, 128, 2, 128), np.float32)
    for c in range(NCH):
        for dst, blk, gn, v0 in ((gw1, 0, g1n, v01[c]), (gw1, 1, g2n, v01[c]),
                                 (gw2, 2, g2n, v02[c]), (gw2, 3, g1n, v02[c])):
            rows = np.minimum(v0 + np.arange(W), G - 1)
            gsel = gn[rows].T.reshape(2, 128, W).transpose(1, 0, 2)
            dst[c, :, :, blk * W:(blk + 1) * W] = gsel
    gw1 = to_f8(gw1)
    gw2 = to_f8(gw2)

    rws_np = np.zeros((NCH, 128, 26), np.float32)
    rwc_np = np.zeros((NCH, 128, 26), np.float32)
    rn1_np = np.zeros((NCH, 128, 2, 32), np.float32)
    rn2_np = np.zeros((NCH, 128, 2, 32), np.float32)
    for c in range(NCH):
        rws_np[c, 0:W, c] = 1.0
        rws_np[c, 2 * W:3 * W, 13 + c] = 1.0
        rwc_np[c, W:2 * W, c] = 1.0
        rwc_np[c, 3 * W:4 * W, 13 + c] = 1.0
        rn1_np[c, :, :, c] = 1.0
        rn2_np[c, :, :, 13 + c] = 1.0

    return {
        "z1t": prep_z(z1p), "z2t": prep_z(z2p),
        "oht": np.ascontiguousarray(oh.astype(f8)),
        "gw1t": np.ascontiguousarray(gw1),
        "gw2t": np.ascontiguousarray(gw2),
        "rws": np.ascontiguousarray(rws_np.astype(bf)),
        "rwc": np.ascontiguousarray(rwc_np.astype(bf)),
        "rwn1": np.ascontiguousarray(rn1_np.astype(f8)),
        "rwn2": np.ascontiguousarray(rn2_np.astype(f8)),
    }


def _prep_inputs(z1, z2, g1, g2, batch_1, batch_2):
    z1 = np.asarray(z1, np.float32)
    z2 = np.asarray(z2, np.float32)
    b1 = np.asarray(batch_1).astype(np.int64).ravel()
    b2 = np.asarray(batch_2).astype(np.int64).ravel()
    g1 = np.asarray(g1, np.float32)
    g2 = np.asarray(g2, np.float32)
    g1n = g1 / np.maximum(np.linalg.norm(g1, axis=1, keepdims=True), 1e-12)
    g2n = g2 / np.maximum(np.linalg.norm(g2, axis=1, keepdims=True), 1e-12)

    in_maps = []
    for k in range(NCORES):
        sl = slice(k * RPC, (k + 1) * RPC)
        in_maps.append(_prep_core(z1[sl], z2[sl], b1[sl], b2[sl], g1n, g2n))
    return in_maps


def _finish(results):
    t1 = 0.0
    t2 = 0.0
    for r in results:
        a = r["acc"].astype(np.float64).ravel()
        t1 += a[0:13].sum()
        t2 += a[13:26].sum()
    return np.float32(np.sqrt(t1) + np.sqrt(t2))


_prog = None


def _get_prog():
    global _prog
    if _prog is None:
        _prog = build()
    return _prog


def kernel(z1, z2, g1, g2, batch_1, batch_2, trace=False):
    nc = _get_prog()
    in_maps = _prep_inputs(z1, z2, g1, g2, batch_1, batch_2)
    res = run_bass_kernel_spmd(nc, in_maps, core_ids=list(range(NCORES)),
                               trace=trace)
    out = _finish(res.results)
    if trace:
        kernel.last_results = res
    return out


# revision 4
# speedup vs baseline: 2.0869x; 1.9555x over previous
"""Bass/Trainium2 kernel v3 for nn_GCL_49959059587771 (GCL JSD loss).

Math (see reference): per node i and pair p with sorted batch index b_i:
    s_i = <z_i, gn_self[b_i]> / ||z_i||
    c_i = <z_i, gn_cross[b_i]> / ||z_i||
    d_i = softplus(-c_i) - softplus(-s_i)
    answer = sqrt(sum d1_i^2) + sqrt(sum d2_i^2)

Strategy:
  - nodes sharded 8 ways (6250/core, padded to 14 chunks of 512)
  - ALL per-chunk inputs (transposed fp8 z, one-hot mask, g-window weights,
    reduce weights) packed into ONE byte tensor, DMA'd in 2-chunk batches
    (~848 KB each) to avoid descriptor/issue overheads
  - per chunk: one DoubleRow fp8 matmul per pair (K=256) computes all
    window sims into one PSUM bank (zero-padded weight halves let both
    pairs accumulate into one bank); one DVE op applies the one-hot mask
    fused with the PSUM evac; one-hot reduce matmuls accumulate s/c rows
    and (from fp8 squared z) norm rows into persistent PSUM banks laid
    out [pair*14 + chunk, 512]
  - short Ln/Exp epilogue; host sums 8x28 partials, sqrt, add.
"""

import numpy as np
from contextlib import ExitStack

import concourse.bass as bass
import concourse.bacc as bacc
import concourse.tile as tile
import concourse.mybir as mybir
from concourse.bass_utils import run_bass_kernel_spmd

N, G, D = 50000, 512, 256
NCORES = 8
RPC = N // NCORES            # 6250 nodes per core
CH = 512                     # nodes per chunk
NCH = 14                     # chunks per core
NODES = NCH * CH             # 7168 padded
W = 32                       # per-chunk batch-value window (per pair)
BG = 2                       # chunks per DMA batch
NB = NCH // BG               # 7 batches

# packed per-chunk per-partition byte layout
OFF_Z1 = 0
OFF_Z2 = 1024
OFF_OH = 2048
OFF_GW1 = 2560
OFF_GW2 = 2816
OFF_RWS = 3072               # bf16 [28]
OFF_RWC = 3128               # bf16 [28]
OFF_RN1 = 3184               # f8 [2, 32]
OFF_RN2 = 3248
CHB = 3312                   # bytes per chunk per partition

AF = mybir.ActivationFunctionType
ALU = mybir.AluOpType
F32 = mybir.dt.float32
BF16 = mybir.dt.bfloat16
F8 = mybir.dt.float8e4
DR = mybir.MatmulPerfMode.DoubleRow

NR = 2 * NCH                 # 28 accumulator rows


def build(debug=False):
    nc = bacc.Bacc("TRN2", target_bir_lowering=False, debug=debug)

    pkt = nc.dram_tensor("pkt", [NB, 128, BG * CHB], F8, kind="ExternalInput")
    acc = nc.dram_tensor("acc", [NR, 1], F32, kind="ExternalOutput")

    with tile.TileContext(nc) as tc, ExitStack() as ctx:
        singles = ctx.enter_context(tc.tile_pool(name="singles", bufs=1))
        pkpool = ctx.enter_context(tc.tile_pool(name="pk", bufs=3))
        sqpool = ctx.enter_context(tc.tile_pool(name="sq", bufs=2))
        mkpool = ctx.enter_context(tc.tile_pool(name="mk", bufs=2))
        ppool = ctx.enter_context(tc.tile_pool(name="pp", bufs=2, space="PSUM"))
        apool = ctx.enter_context(tc.tile_pool(name="acc", bufs=1, space="PSUM"))

        acc_s = apool.tile([NR, CH], F32)
        acc_c = apool.tile([NR, CH], F32)
        acc_n = apool.tile([32, CH], F32)

        eps_b = singles.tile([NR, 1], F32)
        nc.vector.memset(eps_b[:], 1e-12)
        one_b = singles.tile([NR, 1], F32)
        nc.vector.memset(one_b[:], 1.0)
        # preload the natural_log_exp ACT table set early (overlaps DMA)
        dummy = singles.tile([NR, 1], F32)
        nc.scalar.activation(out=dummy[:], in_=eps_b[:], func=AF.Ln, bias=1.0)

        # software-pipelined issue: reduce-MMs for chunk c are issued one
        # chunk later so the PE queue never stalls on this chunk's DVE/ACT
        stage = {}
        pks = [None] * NB
        for c in range(NCH + 1):
            if c < NCH:
                b, cl = divmod(c, BG)
                if cl == 0:
                    pk = pkpool.tile([128, BG * CHB], F8, tag="pk")
                    nc.sync.dma_start(pk[:], pkt[b])
                    pks[b] = pk
                pk = pks[b]
                o = cl * CHB
                z1c = pk[:, o + OFF_Z1:o + OFF_Z1 + 1024].rearrange(
                    "p (k n) -> p k n", k=2)
                z2c = pk[:, o + OFF_Z2:o + OFF_Z2 + 1024].rearrange(
                    "p (k n) -> p k n", k=2)
                ohc = pk[:, o + OFF_OH:o + OFF_OH + 512]
                gw1c = pk[:, o + OFF_GW1:o + OFF_GW1 + 256].rearrange(
                    "p (k n) -> p k n", k=2)
                gw2c = pk[:, o + OFF_GW2:o + OFF_GW2 + 256].rearrange(
                    "p (k n) -> p k n", k=2)
                rwsc = pk[:, o + OFF_RWS:o + OFF_RWS + 56].bitcast(BF16)
                rwcc = pk[:, o + OFF_RWC:o + OFF_RWC + 56].bitcast(BF16)
                rn1c = pk[:, o + OFF_RN1:o + OFF_RN1 + 64].rearrange(
                    "p (k n) -> p k n", k=2)
                rn2c = pk[:, o + OFF_RN2:o + OFF_RN2 + 64].rearrange(
                    "p (k n) -> p k n", k=2)

                # ---- P matmuls (PE, DoubleRow fp8, K=256) ----
                pc = ppool.tile([128, CH], F32, tag="p")
                nc.tensor.matmul(pc[:], gw1c, z1c, start=True, stop=False,
                                 perf_mode=DR, skip_group_check=True)
                nc.tensor.matmul(pc[:], gw2c, z2c, start=False, stop=True,
                                 perf_mode=DR, skip_group_check=True)

                # ---- squares (split ACT / DVE for balance) ----
                zq1 = sqpool.tile([128, 2, CH], F8, tag="q1")
                zq2 = sqpool.tile([128, 2, CH], F8, tag="q2")
                nc.scalar.activation(out=zq1[:], in_=z1c, func=AF.Square)
                if c % 3 == 2:
                    nc.scalar.activation(out=zq2[:], in_=z2c,
                                         func=AF.Square)
                else:
                    nc.vector.scalar_tensor_tensor(
                        out=zq2[:], in0=z2c, scalar=1.0, in1=z2c,
                        op0=ALU.mult, op1=ALU.mult)

                # ---- mask + PSUM evac fused (DVE) ----
                mkc = mkpool.tile([128, CH], BF16, tag="mk")
                nc.vector.scalar_tensor_tensor(
                    out=mkc[:], in0=pc[:], scalar=1.0, in1=ohc,
                    op0=ALU.mult, op1=ALU.mult)

                stage[c] = (rwsc, rwcc, rn1c, rn2c, mkc, zq1, zq2)

            cc = c - 1
            if cc >= 0:
                rwsc, rwcc, rn1c, rn2c, mkc, zq1, zq2 = stage.pop(cc)
                first, last = cc == 0, cc == NCH - 1
                nc.tensor.matmul(acc_s[:], rwsc, mkc[:], start=first,
                                 stop=last, skip_group_check=True)
                nc.tensor.matmul(acc_c[:], rwcc, mkc[:], start=first,
                                 stop=last, skip_group_check=True)
                nc.tensor.matmul(acc_n[:], rn1c, zq1[:], start=first,
                                 stop=False, perf_mode=DR,
                                 skip_group_check=True)
                nc.tensor.matmul(acc_n[:], rn2c, zq2[:], start=False,
                                 stop=last, perf_mode=DR,
                                 skip_group_check=True)

        # ---- epilogue ----
        inv = singles.tile([NR, CH], F32)
        nc.scalar.activation(out=inv[:], in_=acc_n[0:NR, :], func=AF.Ln,
                             bias=eps_b[:])
        nc.scalar.activation(out=inv[:], in_=inv[:], func=AF.Exp, scale=-0.5)
        sp = singles.tile([NR, CH], F32)
        nc.vector.scalar_tensor_tensor(out=sp[:], in0=acc_s[:], scalar=1.0,
                                       in1=inv[:], op0=ALU.mult, op1=ALU.mult)
        cp = singles.tile([NR, CH], F32)
        nc.vector.scalar_tensor_tensor(out=cp[:], in0=acc_c[:], scalar=1.0,
                                       in1=inv[:], op0=ALU.mult, op1=ALU.mult)
        # softplus(-x) = ln(1 + exp(-x)); Exp and Ln share one table set
        nc.scalar.activation(out=sp[:], in_=sp[:], func=AF.Exp, scale=-1.0)
        nc.scalar.activation(out=sp[:], in_=sp[:], func=AF.Ln, bias=one_b[:])
        nc.scalar.activation(out=cp[:], in_=cp[:], func=AF.Exp, scale=-1.0)
        nc.scalar.activation(out=cp[:], in_=cp[:], func=AF.Ln, bias=one_b[:])
        dts = singles.tile([NR, CH], F32)
        nc.vector.scalar_tensor_tensor(out=dts[:], in0=cp[:], scalar=1.0,
                                       in1=sp[:], op0=ALU.mult,
                                       op1=ALU.subtract)
        jnk = singles.tile([NR, CH], F32)
        accq = singles.tile([NR, 1], F32)
        nc.scalar.activation(out=jnk[:], in_=dts[:], func=AF.Square,
                             accum_out=accq[:])
        nc.sync.dma_start(acc[:], accq[:])

    nc.compile()
    return nc


# ---------------------------------------------------------------------------
# host-side prep
# ---------------------------------------------------------------------------

def _prep_core(z1s, z2s, b1s, b2s, g1n, g2n):
    """Build one core's packed input. z*s: [RPC, D] f32; b*s sorted int;
    g*n row-normalized [G, D] f32."""
    import ml_dtypes
    f8 = ml_dtypes.float8_e4m3
    bf = ml_dtypes.bfloat16

    def to_f8(x):
        return np.clip(x, -240.0, 240.0).astype(f8)

    nreal = z1s.shape[0]
    pkt = np.zeros((NCH, 128, CHB), np.uint8)

    def put(c, off, arr_bytes):
        pkt[c, :, off:off + arr_bytes.shape[1]] = arr_bytes

    z1p = np.zeros((NODES, D), np.float32)
    z1p[:nreal] = z1s
    z2p = np.zeros((NODES, D), np.float32)
    z2p[:nreal] = z2s
    # [NCH, 128, 2, CH] transposed fp8 (d = k*128 + p, k outer)
    z1t = to_f8(z1p.reshape(NCH, CH, 2, 128).transpose(0, 3, 2, 1))
    z2t = to_f8(z2p.reshape(NCH, CH, 2, 128).transpose(0, 3, 2, 1))

    v01 = np.zeros(NCH, np.int64)
    v02 = np.zeros(NCH, np.int64)
    for c in range(NCH):
        lo = min(c * CH, nreal - 1)
        hi = min((c + 1) * CH, nreal)
        v01[c] = b1s[lo]
        v02[c] = b2s[lo]
        if hi > c * CH:
            assert b1s[hi - 1] - v01[c] < W, f"chunk {c}: pair1 span"
            assert b2s[hi - 1] - v02[c] < W, f"chunk {c}: pair2 span"

    oh = np.zeros((NCH, 128, CH), np.float32)
    for c in range(NCH):
        hi = min((c + 1) * CH, nreal)
        nn = hi - c * CH
        if nn <= 0:
            continue
        idx = np.arange(nn)
        r1 = (b1s[c * CH:hi] - v01[c]).astype(np.int64)
        r2 = (b2s[c * CH:hi] - v02[c]).astype(np.int64)
        oh[c, r1, idx] = 1.0
        oh[c, W + r1, idx] = 1.0
        oh[c, 2 * W + r2, idx] = 1.0
        oh[c, 3 * W + r2, idx] = 1.0

    gw1 = np.zeros((NCH, 128, 2, 128), np.float32)
    gw2 = np.zeros((NCH, 128, 2, 128), np.float32)
    for c in range(NCH):
        for dst, blk, gn, v0 in ((gw1, 0, g1n, v01[c]), (gw1, 1, g2n, v01[c]),
                                 (gw2, 2, g2n, v02[c]), (gw2, 3, g1n, v02[c])):
            rows = np.minimum(v0 + np.arange(W), G - 1)
            gsel = gn[rows].T.reshape(2, 128, W).transpose(1, 0, 2)
            dst[c, :, :, blk * W:(blk + 1) * W] = gsel

    rws_np = np.zeros((NCH, 128, NR), np.float32)
    rwc_np = np.zeros((NCH, 128, NR), np.float32)
    rn1_np = np.zeros((NCH, 128, 2, 32), np.float32)
    rn2_np = np.zeros((NCH, 128, 2, 32), np.float32)
    for c in range(NCH):
        rws_np[c, 0:W, c] = 1.0
        rws_np[c, 2 * W:3 * W, NCH + c] = 1.0
        rwc_np[c, W:2 * W, c] = 1.0
        rwc_np[c, 3 * W:4 * W, NCH + c] = 1.0
        rn1_np[c, :, :, c] = 1.0
        rn2_np[c, :, :, NCH + c] = 1.0

    for c in range(NCH):
        put(c, OFF_Z1, z1t[c].reshape(128, 1024).view(np.uint8))
        put(c, OFF_Z2, z2t[c].reshape(128, 1024).view(np.uint8))
        put(c, OFF_OH, oh[c].astype(f8).view(np.uint8))
        put(c, OFF_GW1, to_f8(gw1[c]).reshape(128, 256).view(np.uint8))
        put(c, OFF_GW2, to_f8(gw2[c]).reshape(128, 256).view(np.uint8))
        put(c, OFF_RWS, rws_np[c].astype(bf).view(np.uint8))
        put(c, OFF_RWC, rwc_np[c].astype(bf).view(np.uint8))
        put(c, OFF_RN1, rn1_np[c].astype(f8).reshape(128, 64).view(np.uint8))
        put(c, OFF_RN2, rn2_np[c].astype(f8).reshape(128, 64).view(np.uint8))

    pkt = pkt.reshape(NB, BG, 128, CHB).transpose(0, 2, 1, 3).reshape(
        NB, 128, BG * CHB)
    return {"pkt": np.ascontiguousarray(pkt).view(f8)}


def _prep_inputs(z1, z2, g1, g2, batch_1, batch_2):
    z1 = np.asarray(z1, np.float32)
    z2 = np.asarray(z2, np.float32)
    b1 = np.asarray(batch_1).astype(np.int64).ravel()
    b2 = np.asarray(batch_2).astype(np.int64).ravel()
    g1 = np.asarray(g1, np.float32)
    g2 = np.asarray(g2, np.float32)
    g1n = g1 / np.maximum(np.linalg.norm(g1, axis=1, keepdims=True), 1e-12)
    g2n = g2 / np.maximum(np.linalg.norm(g2, axis=1, keepdims=True), 1e-12)

    in_maps = []
    for k in range(NCORES):
        sl = slice(k * RPC, (k + 1) * RPC)
        in_maps.append(_prep_core(z1[sl], z2[sl], b1[sl], b2[sl], g1n, g2n))
    return in_maps


def _finish(results):
    t1 = 0.0
    t2 = 0.0
    for r in results:
        a = r["acc"].astype(np.float64).ravel()
        t1 += a[0:NCH].sum()
        t2 += a[NCH:NR].sum()
    return np.float32(np.sqrt(t1) + np.sqrt(t2))


_prog = None


def _get_prog():
    global _prog
    if _prog is None:
        _prog = build()
    return _prog


def kernel(z1, z2, g1, g2, batch_1, batch_2, trace=False):
    nc = _get_prog()
    in_maps = _prep_inputs(z1, z2, g1, g2, batch_1, batch_2)
    res = run_bass_kernel_spmd(nc, in_maps, core_ids=list(range(NCORES)),
                               trace=trace)
    out = _finish(res.results)
    if trace:
        kernel.last_results = res
    return out


# revision 6
# speedup vs baseline: 2.2365x; 1.0716x over previous
"""Bass/Trainium2 kernel v3 for nn_GCL_49959059587771 (GCL JSD loss).

Math (see reference): per node i and pair p with sorted batch index b_i:
    s_i = <z_i, gn_self[b_i]> / ||z_i||
    c_i = <z_i, gn_cross[b_i]> / ||z_i||
    d_i = softplus(-c_i) - softplus(-s_i)
    answer = sqrt(sum d1_i^2) + sqrt(sum d2_i^2)

Strategy:
  - nodes sharded 8 ways (6250/core, padded to 14 chunks of 512)
  - ALL per-chunk inputs (transposed fp8 z, one-hot mask, g-window weights,
    reduce weights) packed into ONE byte tensor, DMA'd in 2-chunk batches
    (~848 KB each) to avoid descriptor/issue overheads
  - per chunk: one DoubleRow fp8 matmul per pair (K=256) computes all
    window sims into one PSUM bank (zero-padded weight halves let both
    pairs accumulate into one bank); one DVE op applies the one-hot mask
    fused with the PSUM evac; one-hot reduce matmuls accumulate s/c rows
    and (from fp8 squared z) norm rows into persistent PSUM banks laid
    out [pair*14 + chunk, 512]
  - short Ln/Exp epilogue; host sums 8x28 partials, sqrt, add.
"""

import numpy as np
from contextlib import ExitStack

import concourse.bass as bass
import concourse.bacc as bacc
import concourse.tile as tile
import concourse.mybir as mybir
from concourse.bass_utils import run_bass_kernel_spmd

N, G, D = 50000, 512, 256
NCORES = 8
RPC = N // NCORES            # 6250 nodes per core
CH = 512                     # nodes per chunk
NCH = 14                     # chunks per core
NODES = NCH * CH             # 7168 padded
W = 32                       # per-chunk batch-value window (per pair)
BG = 2                       # chunks per DMA batch
NB = NCH // BG               # 7 batches

# packed per-chunk per-partition byte layout
OFF_Z1 = 0
OFF_Z2 = 1024
OFF_OH = 2048
OFF_GW1 = 2560
OFF_GW2 = 2816
OFF_RWS = 3072               # bf16 [28]
OFF_RWC = 3128               # bf16 [28]
OFF_RN1 = 3184               # f8 [2, 32]
OFF_RN2 = 3248
CHB = 3312                   # bytes per chunk per partition

AF = mybir.ActivationFunctionType
ALU = mybir.AluOpType
F32 = mybir.dt.float32
BF16 = mybir.dt.bfloat16
F8 = mybir.dt.float8e4
DR = mybir.MatmulPerfMode.DoubleRow

NR = 2 * NCH                 # 28 accumulator rows


def build(debug=False):
    nc = bacc.Bacc("TRN2", target_bir_lowering=False, debug=debug)

    pkt = nc.dram_tensor("pkt", [128, NCH * CHB], F8, kind="ExternalInput")
    acc = nc.dram_tensor("acc", [NR, 1], F32, kind="ExternalOutput")

    with tile.TileContext(nc) as tc, ExitStack() as ctx:
        singles = ctx.enter_context(tc.tile_pool(name="singles", bufs=1))
        sqpool = ctx.enter_context(tc.tile_pool(name="sq", bufs=2))
        mkpool = ctx.enter_context(tc.tile_pool(name="mk", bufs=2))
        ppool = ctx.enter_context(tc.tile_pool(name="pp", bufs=2, space="PSUM"))
        apool = ctx.enter_context(tc.tile_pool(name="acc", bufs=1, space="PSUM"))

        acc_s = apool.tile([NR, CH], F32)
        acc_c = apool.tile([NR, CH], F32)
        acc_n = apool.tile([32, CH], F32)

        eps_b = singles.tile([NR, 1], F32)
        nc.vector.memset(eps_b[:], 1e-12)
        one_b = singles.tile([NR, 1], F32)
        nc.vector.memset(one_b[:], 1.0)
        # preload the natural_log_exp ACT table set early (overlaps DMA)
        dummy = singles.tile([NR, 1], F32)
        nc.scalar.activation(out=dummy[:], in_=eps_b[:], func=AF.Ln, bias=1.0)

        # whole packed input resident in SBUF; staged DMAs, small first so
        # compute starts early, large later for DMA efficiency
        pk = singles.tile([128, NCH * CHB], F8)
        for b0, b1 in ((0, 1), (1, 2), (2, 4), (4, 7), (7, 10), (10, NCH)):
            nc.sync.dma_start(pk[:, b0 * CHB:b1 * CHB],
                              pkt[:, b0 * CHB:b1 * CHB])

        # software-pipelined issue: reduce-MMs for chunk c are issued one
        # chunk later so the PE queue never stalls on this chunk's DVE/ACT
        stage = {}
        for c in range(NCH + 1):
            if c < NCH:
                o = c * CHB
                z1c = pk[:, o + OFF_Z1:o + OFF_Z1 + 1024].rearrange(
                    "p (k n) -> p k n", k=2)
                z2c = pk[:, o + OFF_Z2:o + OFF_Z2 + 1024].rearrange(
                    "p (k n) -> p k n", k=2)
                ohc = pk[:, o + OFF_OH:o + OFF_OH + 512]
                gw1c = pk[:, o + OFF_GW1:o + OFF_GW1 + 256].rearrange(
                    "p (k n) -> p k n", k=2)
                gw2c = pk[:, o + OFF_GW2:o + OFF_GW2 + 256].rearrange(
                    "p (k n) -> p k n", k=2)
                rwsc = pk[:, o + OFF_RWS:o + OFF_RWS + 56].bitcast(BF16)
                rwcc = pk[:, o + OFF_RWC:o + OFF_RWC + 56].bitcast(BF16)
                rn1c = pk[:, o + OFF_RN1:o + OFF_RN1 + 64].rearrange(
                    "p (k n) -> p k n", k=2)
                rn2c = pk[:, o + OFF_RN2:o + OFF_RN2 + 64].rearrange(
                    "p (k n) -> p k n", k=2)

                # ---- P matmuls (PE, DoubleRow fp8, K=256) ----
                pc = ppool.tile([128, CH], F32, tag="p")
                nc.tensor.matmul(pc[:], gw1c, z1c, start=True, stop=False,
                                 perf_mode=DR, skip_group_check=True)
                nc.tensor.matmul(pc[:], gw2c, z2c, start=False, stop=True,
                                 perf_mode=DR, skip_group_check=True)

                # ---- squares (split ACT / DVE for balance) ----
                zq1 = sqpool.tile([128, 2, CH], F8, tag="q1")
                zq2 = sqpool.tile([128, 2, CH], F8, tag="q2")
                nc.scalar.activation(out=zq1[:], in_=z1c, func=AF.Square)
                if c % 3 == 2:
                    nc.scalar.activation(out=zq2[:], in_=z2c,
                                         func=AF.Square)
                else:
                    nc.vector.scalar_tensor_tensor(
                        out=zq2[:], in0=z2c, scalar=1.0, in1=z2c,
                        op0=ALU.mult, op1=ALU.mult)

                # ---- mask + PSUM evac fused (DVE) ----
                mkc = mkpool.tile([128, CH], BF16, tag="mk")
                nc.vector.scalar_tensor_tensor(
                    out=mkc[:], in0=pc[:], scalar=1.0, in1=ohc,
                    op0=ALU.mult, op1=ALU.mult)

                stage[c] = (rwsc, rwcc, rn1c, rn2c, mkc, zq1, zq2)

            cc = c - 1
            if cc >= 0:
                rwsc, rwcc, rn1c, rn2c, mkc, zq1, zq2 = stage.pop(cc)
                first, last = cc == 0, cc == NCH - 1
                nc.tensor.matmul(acc_s[:], rwsc, mkc[:], start=first,
                                 stop=last, skip_group_check=True)
                nc.tensor.matmul(acc_c[:], rwcc, mkc[:], start=first,
                                 stop=last, skip_group_check=True)
                nc.tensor.matmul(acc_n[:], rn1c, zq1[:], start=first,
                                 stop=False, perf_mode=DR,
                                 skip_group_check=True)
                nc.tensor.matmul(acc_n[:], rn2c, zq2[:], start=False,
                                 stop=last, perf_mode=DR,
                                 skip_group_check=True)

        # ---- epilogue ----
        inv = singles.tile([NR, CH], F32)
        nc.scalar.activation(out=inv[:], in_=acc_n[0:NR, :], func=AF.Ln,
                             bias=eps_b[:])
        nc.scalar.activation(out=inv[:], in_=inv[:], func=AF.Exp, scale=-0.5)
        sp = singles.tile([NR, CH], F32)
        nc.vector.scalar_tensor_tensor(out=sp[:], in0=acc_s[:], scalar=1.0,
                                       in1=inv[:], op0=ALU.mult, op1=ALU.mult)
        cp = singles.tile([NR, CH], F32)
        nc.vector.scalar_tensor_tensor(out=cp[:], in0=acc_c[:], scalar=1.0,
                                       in1=inv[:], op0=ALU.mult, op1=ALU.mult)
        # softplus(-x) = ln(1 + exp(-x)); Exp and Ln share one table set
        nc.scalar.activation(out=sp[:], in_=sp[:], func=AF.Exp, scale=-1.0)
        nc.scalar.activation(out=sp[:], in_=sp[:], func=AF.Ln, bias=one_b[:])
        nc.scalar.activation(out=cp[:], in_=cp[:], func=AF.Exp, scale=-1.0)
        nc.scalar.activation(out=cp[:], in_=cp[:], func=AF.Ln, bias=one_b[:])
        dts = singles.tile([NR, CH], F32)
        nc.vector.scalar_tensor_tensor(out=dts[:], in0=cp[:], scalar=1.0,
                                       in1=sp[:], op0=ALU.mult,
                                       op1=ALU.subtract)
        jnk = singles.tile([NR, CH], F32)
        accq = singles.tile([NR, 1], F32)
        nc.vector.scalar_tensor_tensor(out=jnk[:], in0=dts[:], scalar=1.0,
                                       in1=dts[:], op0=ALU.mult, op1=ALU.mult,
                                       accum_out=accq[:])
        nc.sync.dma_start(acc[:], accq[:])

    nc.compile()
    return nc


# ---------------------------------------------------------------------------
# host-side prep
# ---------------------------------------------------------------------------

def _prep_core(z1s, z2s, b1s, b2s, g1n, g2n):
    """Build one core's packed input. z*s: [RPC, D] f32; b*s sorted int;
    g*n row-normalized [G, D] f32."""
    import ml_dtypes
    f8 = ml_dtypes.float8_e4m3
    bf = ml_dtypes.bfloat16

    def to_f8(x):
        return np.clip(x, -240.0, 240.0).astype(f8)

    nreal = z1s.shape[0]
    pkt = np.zeros((NCH, 128, CHB), np.uint8)

    def put(c, off, arr_bytes):
        pkt[c, :, off:off + arr_bytes.shape[1]] = arr_bytes

    z1p = np.zeros((NODES, D), np.float32)
    z1p[:nreal] = z1s
    z2p = np.zeros((NODES, D), np.float32)
    z2p[:nreal] = z2s
    # [NCH, 128, 2, CH] transposed fp8 (d = k*128 + p, k outer)
    z1t = to_f8(z1p.reshape(NCH, CH, 2, 128).transpose(0, 3, 2, 1))
    z2t = to_f8(z2p.reshape(NCH, CH, 2, 128).transpose(0, 3, 2, 1))

    v01 = np.zeros(NCH, np.int64)
    v02 = np.zeros(NCH, np.int64)
    for c in range(NCH):
        lo = min(c * CH, nreal - 1)
        hi = min((c + 1) * CH, nreal)
        v01[c] = b1s[lo]
        v02[c] = b2s[lo]
        if hi > c * CH:
            assert b1s[hi - 1] - v01[c] < W, f"chunk {c}: pair1 span"
            assert b2s[hi - 1] - v02[c] < W, f"chunk {c}: pair2 span"

    oh = np.zeros((NCH, 128, CH), np.float32)
    for c in range(NCH):
        hi = min((c + 1) * CH, nreal)
        nn = hi - c * CH
        if nn <= 0:
            continue
        idx = np.arange(nn)
        r1 = (b1s[c * CH:hi] - v01[c]).astype(np.int64)
        r2 = (b2s[c * CH:hi] - v02[c]).astype(np.int64)
        oh[c, r1, idx] = 1.0
        oh[c, W + r1, idx] = 1.0
        oh[c, 2 * W + r2, idx] = 1.0
        oh[c, 3 * W + r2, idx] = 1.0

    gw1 = np.zeros((NCH, 128, 2, 128), np.float32)
    gw2 = np.zeros((NCH, 128, 2, 128), np.float32)
    for c in range(NCH):
        for dst, blk, gn, v0 in ((gw1, 0, g1n, v01[c]), (gw1, 1, g2n, v01[c]),
                                 (gw2, 2, g2n, v02[c]), (gw2, 3, g1n, v02[c])):
            rows = np.minimum(v0 + np.arange(W), G - 1)
            gsel = gn[rows].T.reshape(2, 128, W).transpose(1, 0, 2)
            dst[c, :, :, blk * W:(blk + 1) * W] = gsel

    rws_np = np.zeros((NCH, 128, NR), np.float32)
    rwc_np = np.zeros((NCH, 128, NR), np.float32)
    rn1_np = np.zeros((NCH, 128, 2, 32), np.float32)
    rn2_np = np.zeros((NCH, 128, 2, 32), np.float32)
    for c in range(NCH):
        rws_np[c, 0:W, c] = 1.0
        rws_np[c, 2 * W:3 * W, NCH + c] = 1.0
        rwc_np[c, W:2 * W, c] = 1.0
        rwc_np[c, 3 * W:4 * W, NCH + c] = 1.0
        rn1_np[c, :, :, c] = 1.0
        rn2_np[c, :, :, NCH + c] = 1.0

    for c in range(NCH):
        put(c, OFF_Z1, z1t[c].reshape(128, 1024).view(np.uint8))
        put(c, OFF_Z2, z2t[c].reshape(128, 1024).view(np.uint8))
        put(c, OFF_OH, oh[c].astype(f8).view(np.uint8))
        put(c, OFF_GW1, to_f8(gw1[c]).reshape(128, 256).view(np.uint8))
        put(c, OFF_GW2, to_f8(gw2[c]).reshape(128, 256).view(np.uint8))
        put(c, OFF_RWS, rws_np[c].astype(bf).view(np.uint8))
        put(c, OFF_RWC, rwc_np[c].astype(bf).view(np.uint8))
        put(c, OFF_RN1, rn1_np[c].astype(f8).reshape(128, 64).view(np.uint8))
        put(c, OFF_RN2, rn2_np[c].astype(f8).reshape(128, 64).view(np.uint8))

    pkt = pkt.transpose(1, 0, 2).reshape(128, NCH * CHB)
    return {"pkt": np.ascontiguousarray(pkt).view(f8)}


def _prep_inputs(z1, z2, g1, g2, batch_1, batch_2):
    z1 = np.asarray(z1, np.float32)
    z2 = np.asarray(z2, np.float32)
    b1 = np.asarray(batch_1).astype(np.int64).ravel()
    b2 = np.asarray(batch_2).astype(np.int64).ravel()
    g1 = np.asarray(g1, np.float32)
    g2 = np.asarray(g2, np.float32)
    g1n = g1 / np.maximum(np.linalg.norm(g1, axis=1, keepdims=True), 1e-12)
    g2n = g2 / np.maximum(np.linalg.norm(g2, axis=1, keepdims=True), 1e-12)

    in_maps = []
    for k in range(NCORES):
        sl = slice(k * RPC, (k + 1) * RPC)
        in_maps.append(_prep_core(z1[sl], z2[sl], b1[sl], b2[sl], g1n, g2n))
    return in_maps


def _finish(results):
    t1 = 0.0
    t2 = 0.0
    for r in results:
        a = r["acc"].astype(np.float64).ravel()
        t1 += a[0:NCH].sum()
        t2 += a[NCH:NR].sum()
    return np.float32(np.sqrt(t1) + np.sqrt(t2))


_prog = None


def _get_prog():
    global _prog
    if _prog is None:
        _prog = build()
    return _prog


def kernel(z1, z2, g1, g2, batch_1, batch_2, trace=False):
    nc = _get_prog()
    in_maps = _prep_inputs(z1, z2, g1, g2, batch_1, batch_2)
    res = run_bass_kernel_spmd(nc, in_maps, core_ids=list(range(NCORES)),
                               trace=trace)
    out = _finish(res.results)
    if trace:
        kernel.last_results = res
    return out
